# revision 4
# baseline (speedup 1.0000x reference)
"""GaussianAdapter Trainium2 kernel: 8-core data-parallel (one camera per core).

Layout per core: 65536 gaussians as [128 partitions x 512 columns],
gaussian g = p*512 + j.  Raw AoS records stream in per 64-column block;
SH masking happens on the strided record view writing packed output;
geometry (quat->cov, rays->means) runs on 256-column half planes.
"""
import numpy as np
from contextlib import ExitStack

import concourse.bass as bass
import concourse.tile as tile
from concourse import bacc, mybir
from concourse.bass_utils import run_bass_kernel_spmd

F32 = mybir.dt.float32
AF = mybir.ActivationFunctionType
OP = mybir.AluOpType

P = 128
COLS = 512          # gaussians per partition per core
W = 64              # phase-1 block width (columns)
NBLK = COLS // W
HALF = 256          # phase-2 geometry width
D_IN = 82
D_SH = 75

# cam vector layout (per-partition replicated constants)
C_AX, C_BX, C_AY, C_BY, C_M145, C_M05 = 0, 1, 2, 3, 4, 5
C_R = 6            # 6..14  c2w rotation row-major
C_T = 15           # 15..17 translation
C_NR = 18          # 18..26 negated rotation entries
NCAM = 27

# engine split knobs
SH_G_BLOCKS = {2, 5}          # phase-1 SH-multiply blocks done on gpsimd
COVOFF_ON_G = True            # off-diagonal covariance products on gpsimd


def _build():
    nc = bacc.Bacc("TRN2", target_bir_lowering=False, debug=False, num_devices=8)

    raw = nc.dram_tensor("raw", [P, COLS, D_IN], F32, kind="ExternalInput").ap()
    crd = nc.dram_tensor("crd", [P, COLS, 2], F32, kind="ExternalInput").ap()
    dep = nc.dram_tensor("dep", [P, COLS], F32, kind="ExternalInput").ap()
    cam = nc.dram_tensor("cam", [P, NCAM], F32, kind="ExternalInput").ap()
    msk = nc.dram_tensor("msk", [P, D_SH], F32, kind="ExternalInput").ap()

    o_sh = nc.dram_tensor("o_sh", [P, COLS, D_SH], F32, kind="ExternalOutput").ap()
    o_mean = nc.dram_tensor("o_mean", [P, COLS, 3], F32, kind="ExternalOutput").ap()
    o_cov = nc.dram_tensor("o_cov", [P, COLS, 9], F32, kind="ExternalOutput").ap()
    o_scl = nc.dram_tensor("o_scl", [P, COLS, 3], F32, kind="ExternalOutput").ap()
    o_rot = nc.dram_tensor("o_rot", [P, COLS, 4], F32, kind="ExternalOutput").ap()

    with tile.TileContext(nc) as tc, ExitStack() as ctx:
        persist = ctx.enter_context(tc.tile_pool(name="persist", bufs=1))
        rawp = ctx.enter_context(tc.tile_pool(name="rawp", bufs=2))
        shp = ctx.enter_context(tc.tile_pool(name="shp", bufs=2))
        tmp = ctx.enter_context(tc.tile_pool(name="tmp", bufs=1))
        chain = ctx.enter_context(tc.tile_pool(name="chain", bufs=2))
        outs = ctx.enter_context(tc.tile_pool(name="outs", bufs=2))

        V, A, G, S = nc.vector, nc.scalar, nc.gpsimd, nc.sync

        t_cam = persist.tile([P, NCAM], F32)
        S.dma_start(out=t_cam[:], in_=cam[:])
        t_msk = persist.tile([P, D_SH], F32)
        S.dma_start(out=t_msk[:], in_=msk[:])
        t_dep = persist.tile([P, COLS], F32)
        S.dma_start(out=t_dep[:], in_=dep[:])
        t_crd = persist.tile([P, COLS, 2], F32)
        S.dma_start(out=t_crd[:], in_=crd[:])

        def camv(idx):
            return t_cam[:, idx:idx + 1]

        # per-half staging of quaternions / sigmoid outputs
        q_h = [persist.tile([P, HALF, 4], F32, tag=f"qh{h}", name=f"qh{h}") for h in range(2)]
        sg_h = [persist.tile([P, HALF, 3], F32, tag=f"sgh{h}", name=f"sgh{h}") for h in range(2)]

        m_ap = t_msk[:]

        # ---------------- phase 1: stream AoS blocks ----------------
        for b in range(NBLK):
            cs = slice(b * W, (b + 1) * W)
            h = (b * W) // HALF
            ls = slice(b * W - h * HALF, (b + 1) * W - h * HALF)

            T = rawp.tile([P, W, D_IN], F32, tag="T", name="T")
            S.dma_start(out=T[:], in_=raw[:, cs, :])

            sh_t = shp.tile([P, W, D_SH], F32, tag="sh", name="sh")
            mask_b = bass.AP(
                tensor=m_ap.tensor, offset=m_ap.offset,
                ap=[m_ap.ap[0], [0, W], m_ap.ap[1]],
            )
            sh_eng = G if b in SH_G_BLOCKS else V
            sh_eng.tensor_tensor(out=sh_t[:], in0=T[:, :, 7:82], in1=mask_b,
                                 op=OP.mult)
            S.dma_start(out=o_sh[:, cs, :], in_=sh_t[:])

            A.copy(out=q_h[h][:, ls, :], in_=T[:, :, 3:7])
            A.activation(out=sg_h[h][:, ls, :], in_=T[:, :, 0:3], func=AF.Sigmoid)

        # ---------------- phase 2: geometry per half ----------------
        for h in range(2):
            cc = slice(h * HALF, (h + 1) * HALF)

            def tp(tag):
                return tmp.tile([P, HALF], F32, tag=tag, name=tag)

            q = [q_h[h][:, :, f] for f in range(4)]   # r, i, j, k (stride-4)

            # squares of quaternion components (ACT)
            sq = [tp("sq0"), tp("sq1"), tp("sq2"), tp("sq3")]
            for f in range(4):
                A.activation(out=sq[f], in_=q[f], func=AF.Square)

            ssb = tp("ssb")
            V.tensor_add(sq[0], sq[0], sq[1])          # sq0 <- rr+ii (rr only needed here)
            V.tensor_add(ssb, sq[2], sq[3])
            V.tensor_add(ssb, sq[0], ssb)              # ssb <- |q|^2

            inv, rinv, t2 = tp("inv"), tp("rinv"), tp("t2")
            V.reciprocal_approx_fast(out=inv, in_=ssb)
            A.activation(out=rinv, in_=inv, func=AF.Sqrt)
            V.tensor_scalar_mul(t2, inv, 2.0)

            rot_t = outs.tile([P, HALF, 4], F32, tag="rot", name="rot")
            for f in range(4):
                V.tensor_mul(rot_t[:, :, f], q[f], rinv)

            # cross products (i=q1, j=q2, k=q3, r=q0)
            ij, ik, jk = tp("ij"), tp("ik"), tp("jk")
            ri, rj, rk = tp("ri"), tp("rj"), tp("rk")
            V.tensor_mul(ij, q[1], q[2])
            V.tensor_mul(ik, q[1], q[3])
            V.tensor_mul(jk, q[2], q[3])
            V.tensor_mul(ri, q[0], q[1])
            V.tensor_mul(rj, q[0], q[2])
            V.tensor_mul(rk, q[0], q[3])

            # pair sums / diffs (gpsimd)
            d = [tp("d0"), tp("d1"), tp("d2")]
            G.tensor_add(d[0], sq[2], sq[3])      # jj+kk
            G.tensor_add(d[1], sq[1], sq[3])      # ii+kk
            G.tensor_add(d[2], sq[1], sq[2])      # ii+jj
            o1, o2 = tp("o1"), tp("o2")
            o3, o4 = tp("o3"), tp("o4")
            o5, o6 = tp("o5"), tp("o6")
            G.tensor_sub(o1, ij, rk)
            G.tensor_add(o2, ij, rk)
            G.tensor_add(o3, ik, rj)
            G.tensor_sub(o4, ik, rj)
            G.tensor_sub(o5, jk, ri)
            G.tensor_add(o6, jk, ri)
            dm = d
            for jx in range(3):
                G.tensor_mul(dm[jx], d[jx], t2)   # two_s*(…): 1 - dm = Rm[j][j]

            # off-diagonal rotation-matrix entries (x two_s), in place over o*
            for t_o in (o1, o2, o3, o4, o5, o6):
                V.tensor_mul(t_o, t_o, t2)
            # Rm[r][c] for r != c; diag handled via dm in the G chains
            Roff = {(0, 1): o1, (0, 2): o3, (1, 0): o2,
                    (1, 2): o5, (2, 0): o4, (2, 1): o6}

            # scales -> scl_t (and kept for H)
            scl_t = outs.tile([P, HALF, 3], F32, tag="scl", name="scl")
            u = [tp("u0"), tp("u1"), tp("u2")]
            for k in range(3):
                A.activation(out=u[k], in_=sg_h[h][:, :, k], func=AF.Copy,
                             scale=camv(C_M145))
                V.scalar_tensor_tensor(out=scl_t[:, :, k], in0=u[k],
                                       scalar=camv(C_M05), in1=t_dep[:, cc],
                                       op0=OP.add, op1=OP.mult)

            # H = c2w · Rm · diag(s)
            H = [[None] * 3 for _ in range(3)]
            for i in range(3):
                for j in range(3):
                    g0 = chain.tile([P, HALF], F32, tag="g0", name="g0")
                    # diag term: c_ij*(1 - dm_j) = (-c_ij)*dm_j + c_ij
                    A.activation(out=g0, in_=dm[j], func=AF.Identity,
                                 scale=camv(C_NR + 3 * i + j),
                                 bias=camv(C_R + 3 * i + j))
                    g1 = chain.tile([P, HALF], F32, tag="g1", name="g1")
                    k1, k2 = [k for k in range(3) if k != j]
                    V.scalar_tensor_tensor(out=g1, in0=Roff[(k1, j)],
                                           scalar=camv(C_R + 3 * i + k1),
                                           in1=g0, op0=OP.mult, op1=OP.add)
                    g2 = chain.tile([P, HALF], F32, tag="g2", name="g2")
                    V.scalar_tensor_tensor(out=g2, in0=Roff[(k2, j)],
                                           scalar=camv(C_R + 3 * i + k2),
                                           in1=g1, op0=OP.mult, op1=OP.add)
                    Hij = tmp.tile([P, HALF], F32, tag=f"H{i}{j}", name=f"H{i}{j}")
                    V.tensor_mul(Hij, g2, scl_t[:, :, j])
                    H[i][j] = Hij

            cov_t = outs.tile([P, HALF, 9], F32, tag="cov", name="cov")
            p0, p1, p2, s01 = tp("p0"), tp("p1"), tp("p2"), tp("s01")
            # diagonal: sum of squares of H row i
            for i in range(3):
                for m, pm in ((0, p0), (1, p1), (2, p2)):
                    A.activation(out=pm, in_=H[i][m], func=AF.Square)
                V.tensor_add(s01, p0, p1)
                V.tensor_add(cov_t[:, :, 4 * i], s01, p2)
            # off-diagonal
            pe = G if COVOFF_ON_G else V
            for (i, l) in ((0, 1), (0, 2), (1, 2)):
                pe.tensor_mul(p0, H[i][0], H[l][0])
                pe.tensor_mul(p1, H[i][1], H[l][1])
                pe.tensor_mul(p2, H[i][2], H[l][2])
                pe.tensor_add(s01, p0, p1)
                pe.tensor_add(cov_t[:, :, 3 * i + l], s01, p2)
                A.copy(out=cov_t[:, :, 3 * l + i], in_=cov_t[:, :, 3 * i + l])

            # means
            dx, dy = tp("dx"), tp("dy")
            V.tensor_scalar(out=dx, in0=t_crd[:, cc, 0], scalar1=camv(C_AX),
                            scalar2=camv(C_BX), op0=OP.mult, op1=OP.add)
            V.tensor_scalar(out=dy, in0=t_crd[:, cc, 1], scalar1=camv(C_AY),
                            scalar2=camv(C_BY), op0=OP.mult, op1=OP.add)
            sx, sy, dd = tp("sx"), tp("sy"), tp("dd")
            A.activation(out=sx, in_=dx, func=AF.Square)
            A.activation(out=sy, in_=dy, func=AF.Square)
            V.tensor_add(dd, sx, sy)
            nrm, rr, dsc = tp("nrm"), tp("rr"), tp("dsc")
            A.activation(out=nrm, in_=dd, func=AF.Sqrt, bias=1.0)
            V.reciprocal_approx_fast(out=rr, in_=nrm)
            V.tensor_mul(dsc, rr, t_dep[:, cc])

            mean_t = outs.tile([P, HALF, 3], F32, tag="mean", name="mean")
            for i in range(3):
                w1 = chain.tile([P, HALF], F32, tag="w1", name="w1")
                A.activation(out=w1, in_=dx, func=AF.Copy,
                             scale=camv(C_R + 3 * i))
                w2 = chain.tile([P, HALF], F32, tag="w2", name="w2")
                V.scalar_tensor_tensor(out=w2, in0=dy,
                                       scalar=camv(C_R + 3 * i + 1), in1=w1,
                                       op0=OP.mult, op1=OP.add)
                w3 = chain.tile([P, HALF], F32, tag="w3", name="w3")
                V.scalar_tensor_tensor(out=w3, in0=w2,
                                       scalar=camv(C_R + 3 * i + 2), in1=dsc,
                                       op0=OP.add, op1=OP.mult)
                V.tensor_scalar(out=mean_t[:, :, i], in0=w3,
                                scalar1=camv(C_T + i), scalar2=None,
                                op0=OP.add)

            S.dma_start(out=o_mean[:, cc, :], in_=mean_t[:])
            S.dma_start(out=o_cov[:, cc, :], in_=cov_t[:])
            S.dma_start(out=o_scl[:, cc, :], in_=scl_t[:])
            S.dma_start(out=o_rot[:, cc, :], in_=rot_t[:])

    nc.compile()
    return nc


_NC = None


def _get_nc():
    global _NC
    if _NC is None:
        _NC = _build()
    return _NC


def _sh_mask75():
    m = np.ones(25, np.float32)
    for dg in range(1, 5):
        m[dg * dg:(dg + 1) * (dg + 1)] = np.float32(0.1 * 0.25 ** dg)
    return np.tile(m, 3)


def kernel(extrinsics, intrinsics, coordinates, depths, opacities,
           raw_gaussians, h=256, w=256, eps=1e-8):
    nc = _get_nc()
    hh, ww = int(h), int(w)
    ext = np.asarray(extrinsics, dtype=np.float32)
    K = np.asarray(intrinsics, dtype=np.float32)
    crd = np.asarray(coordinates, dtype=np.float32)
    dth = np.asarray(depths, dtype=np.float32)
    opa = np.asarray(opacities, dtype=np.float32)
    rg = np.asarray(raw_gaussians, dtype=np.float32)

    msk = np.broadcast_to(_sh_mask75(), (P, D_SH)).copy()

    in_maps = []
    for c in range(8):
        b, v = c // 2, c % 2
        Kb = K[b, v, 0].astype(np.float64)
        fx, fy, cx, cy = Kb[0, 0], Kb[1, 1], Kb[0, 2], Kb[1, 2]
        mult = 0.1 * (1.0 / (fx * ww) + 1.0 / (fy * hh))
        Rm = ext[b, v, 0, :3, :3].astype(np.float64)
        t = ext[b, v, 0, :3, 3].astype(np.float64)
        camv = np.concatenate([
            [1.0 / fx, -cx / fx, 1.0 / fy, -cy / fy, 14.5 * mult, 0.5 * mult],
            Rm.flatten(), t, -Rm.flatten(),
        ]).astype(np.float32)
        in_maps.append({
            "raw": np.ascontiguousarray(rg[b, v]).reshape(P, COLS, D_IN),
            "crd": np.ascontiguousarray(crd[b, v]).reshape(P, COLS, 2),
            "dep": np.ascontiguousarray(dth[b, v]).reshape(P, COLS),
            "cam": np.broadcast_to(camv, (P, NCAM)).copy(),
            "msk": msk,
        })

    res = run_bass_kernel_spmd(nc, in_maps, list(range(8))).results

    def gather(name, klast):
        return np.stack([res[c][name].reshape(65536, klast) for c in range(8)]
                        ).reshape(4, 2, 65536, klast)

    means = gather("o_mean", 3)
    cov = gather("o_cov", 9).reshape(4, 2, 65536, 3, 3)
    scales = gather("o_scl", 3)
    rotations = gather("o_rot", 4)
    sh = gather("o_sh", D_SH).reshape(4, 2, 65536, 3, 25)
    return means, cov, scales, rotations, sh, opa


# revision 28
# speedup vs baseline: 1.7996x; 1.7996x over previous
"""GaussianAdapter Trainium2 kernel: 8-core data-parallel (one camera per core).

Layout per core: 65536 gaussians as [128 partitions x 512 columns],
gaussian g = p*512 + j.  Raw AoS records stream in per 64-column block;
SH masking happens on the strided record view writing packed output;
geometry (quat->cov, rays->means) runs on 256-column half planes.
"""
import numpy as np
from contextlib import ExitStack

import concourse.bass as bass
import concourse.tile as tile
from concourse import bacc, mybir
from concourse.bass_utils import run_bass_kernel_spmd

F32 = mybir.dt.float32
BF16 = mybir.dt.bfloat16
AF = mybir.ActivationFunctionType
OP = mybir.AluOpType

P = 128
COLS = 512          # gaussians per partition per core
W = 64              # phase-1 block width (columns)
NBLK = COLS // W
HALF = 256          # phase-2 geometry width (first chunk)
CHUNKS = [(0, 256), (256, 128), (384, 128)]
D_IN = 82
D_SH = 75

# cam vector layout (per-partition replicated constants)
C_AX, C_BX, C_AY, C_BY, C_M145, C_M05 = 0, 1, 2, 3, 4, 5
C_R = 6            # 6..14  c2w rotation row-major
C_T = 15           # 15..17 translation
C_NR = 18          # 18..26 negated rotation entries
NCAM = 27

# engine split knobs
SH_G_BLOCKS = set()          # phase-1 SH-multiply blocks done on gpsimd
COVOFF_ON_G = True            # off-diagonal covariance products on gpsimd


def _build():
    nc = bacc.Bacc("TRN2", target_bir_lowering=False, debug=False, num_devices=8)

    rawg = nc.dram_tensor("rawg", [P, COLS, 7], F32, kind="ExternalInput").ap()
    raws = nc.dram_tensor("raws", [P, COLS, D_SH], BF16, kind="ExternalInput").ap()
    crd = nc.dram_tensor("crd", [P, COLS, 2], F32, kind="ExternalInput").ap()
    dep = nc.dram_tensor("dep", [P, COLS], F32, kind="ExternalInput").ap()
    cam = nc.dram_tensor("cam", [P, NCAM], F32, kind="ExternalInput").ap()
    msk = nc.dram_tensor("msk", [P, 2 * D_SH], BF16, kind="ExternalInput").ap()

    o_sh = nc.dram_tensor("o_sh", [P, COLS, D_SH], BF16, kind="ExternalOutput").ap()
    o_mean = nc.dram_tensor("o_mean", [P, COLS, 3], BF16, kind="ExternalOutput").ap()
    o_cov = nc.dram_tensor("o_cov", [P, COLS, 9], BF16, kind="ExternalOutput").ap()
    o_scl = nc.dram_tensor("o_scl", [P, COLS, 3], BF16, kind="ExternalOutput").ap()
    o_rot = nc.dram_tensor("o_rot", [P, COLS, 4], BF16, kind="ExternalOutput").ap()

    with tile.TileContext(nc) as tc, ExitStack() as ctx:
        persist = ctx.enter_context(tc.tile_pool(name="persist", bufs=1))
        rawp = ctx.enter_context(tc.tile_pool(name="rawp", bufs=4))
        shp = ctx.enter_context(tc.tile_pool(name="shp", bufs=3))
        tmp = ctx.enter_context(tc.tile_pool(name="tmp", bufs=1))
        chain = ctx.enter_context(tc.tile_pool(name="chain", bufs=2))
        outs = ctx.enter_context(tc.tile_pool(name="outs", bufs=3))

        V, A, G, S = nc.vector, nc.scalar, nc.gpsimd, nc.sync
        SO = nc.sync  # output DMAs ring

        t_cam = persist.tile([P, NCAM], F32)
        S.dma_start(out=t_cam[:], in_=cam[:])
        t_msk = persist.tile([P, 2 * D_SH], BF16)
        S.dma_start(out=t_msk[:], in_=msk[:])
        t_dep = persist.tile([P, COLS], F32)
        S.dma_start(out=t_dep[:], in_=dep[:])
        t_crd = persist.tile([P, COLS, 2], F32)
        S.dma_start(out=t_crd[:], in_=crd[:])

        def camv(idx):
            return t_cam[:, idx:idx + 1]

        # per-chunk staging of quaternions / sigmoid outputs
        q_h = [persist.tile([P, cw, 4], F32, tag=f"qh{ci}", name=f"qh{ci}")
               for ci, (c0, cw) in enumerate(CHUNKS)]
        sg_h = [persist.tile([P, cw, 3], F32, tag=f"sgh{ci}", name=f"sgh{ci}")
                for ci, (c0, cw) in enumerate(CHUNKS)]

        m_ap = t_msk[:]

        # ---------------- phase 2: geometry per chunk ----------------
        def geometry_chunk(h):
            c0, cw = CHUNKS[h]
            lastc = False  # last-chunk-on-DVE: measured net loss
            cc = slice(c0, c0 + cw)

            DBUF = {"sq0", "sq1", "sq2", "sq3", "ssb", "inv", "rinv", "t2",
                    "ij", "ik", "jk", "ri", "rj", "rk", "d0", "d1", "d2",
                    "o1", "o2", "o3", "o4", "o5", "o6"}

            def tp(tag):
                return tmp.tile([P, cw], F32, tag=tag, name=tag,
                                bufs=2 if tag in DBUF else 1)

            q = [q_h[h][:, :, f] for f in range(4)]   # r, i, j, k (stride-4)
            EG = V if lastc else G       # minimise cross-engine hops at the tail

            # squares of quaternion components
            sq = [tp("sq0"), tp("sq1"), tp("sq2"), tp("sq3")]
            for f in range(4):
                if lastc:
                    V.tensor_mul(sq[f], q[f], q[f])
                else:
                    A.activation(out=sq[f], in_=q[f], func=AF.Square)

            ssb = tp("ssb")
            V.tensor_add(sq[0], sq[0], sq[1])          # sq0 <- rr+ii (rr only needed here)
            V.tensor_add(ssb, sq[2], sq[3])
            V.tensor_add(ssb, sq[0], ssb)              # ssb <- |q|^2

            inv, rinv, t2 = tp("inv"), tp("rinv"), tp("t2")
            V.reciprocal_approx_fast(out=inv, in_=ssb)
            A.activation(out=rinv, in_=inv, func=AF.Sqrt)
            V.tensor_scalar_mul(t2, inv, 2.0)

            rot_t = outs.tile([P, cw, 4], BF16, tag="rot", name="rot")
            for f in range(4):
                EG.tensor_mul(rot_t[:, :, f], q[f], rinv)

            # cross products (i=q1, j=q2, k=q3, r=q0)
            ij, ik, jk = tp("ij"), tp("ik"), tp("jk")
            ri, rj, rk = tp("ri"), tp("rj"), tp("rk")
            V.tensor_mul(ij, q[1], q[2])
            V.tensor_mul(ik, q[1], q[3])
            V.tensor_mul(jk, q[2], q[3])
            V.tensor_mul(ri, q[0], q[1])
            V.tensor_mul(rj, q[0], q[2])
            V.tensor_mul(rk, q[0], q[3])

            # pair sums / diffs (gpsimd)
            d = [tp("d0"), tp("d1"), tp("d2")]
            EG.tensor_add(d[0], sq[2], sq[3])      # jj+kk
            EG.tensor_add(d[1], sq[1], sq[3])      # ii+kk
            EG.tensor_add(d[2], sq[1], sq[2])      # ii+jj
            o1, o2 = tp("o1"), tp("o2")
            o3, o4 = tp("o3"), tp("o4")
            o5, o6 = tp("o5"), tp("o6")
            EG.tensor_sub(o1, ij, rk)
            EG.tensor_add(o2, ij, rk)
            EG.tensor_add(o3, ik, rj)
            EG.tensor_sub(o4, ik, rj)
            EG.tensor_sub(o5, jk, ri)
            EG.tensor_add(o6, jk, ri)
            dm = d
            for jx in range(3):
                EG.tensor_mul(dm[jx], d[jx], t2)   # two_s*(…): 1 - dm = Rm[j][j]

            # off-diagonal rotation-matrix entries (x two_s), in place over o*
            for t_o in (o1, o2, o3, o4, o5, o6):
                V.tensor_mul(t_o, t_o, t2)
            # Rm[r][c] for r != c; diag handled via dm in the G chains
            Roff = {(0, 1): o1, (0, 2): o3, (1, 0): o2,
                    (1, 2): o5, (2, 0): o4, (2, 1): o6}

            # scales -> scl_t (and kept for H)
            scl_t = outs.tile([P, cw, 3], BF16, tag="scl", name="scl")
            u = [tp("u0"), tp("u1"), tp("u2")]
            for k in range(3):
                A.activation(out=u[k], in_=sg_h[h][:, :, k], func=AF.Copy,
                             scale=camv(C_M145))
                V.scalar_tensor_tensor(out=scl_t[:, :, k], in0=u[k],
                                       scalar=camv(C_M05), in1=t_dep[:, cc],
                                       op0=OP.add, op1=OP.mult)

            # H = c2w · Rm · diag(s)
            H = [[None] * 3 for _ in range(3)]
            for i in range(3):
                for j in range(3):
                    g0 = chain.tile([P, cw], F32, tag="g0", name="g0")
                    # diag term: c_ij*(1 - dm_j) = (-c_ij)*dm_j + c_ij
                    if lastc:
                        V.tensor_scalar(out=g0, in0=dm[j],
                                        scalar1=camv(C_NR + 3 * i + j),
                                        scalar2=camv(C_R + 3 * i + j),
                                        op0=OP.mult, op1=OP.add)
                    else:
                        A.activation(out=g0, in_=dm[j], func=AF.Identity,
                                     scale=camv(C_NR + 3 * i + j),
                                     bias=camv(C_R + 3 * i + j))
                    g1 = chain.tile([P, cw], F32, tag="g1", name="g1")
                    k1, k2 = [k for k in range(3) if k != j]
                    V.scalar_tensor_tensor(out=g1, in0=Roff[(k1, j)],
                                           scalar=camv(C_R + 3 * i + k1),
                                           in1=g0, op0=OP.mult, op1=OP.add)
                    g2 = chain.tile([P, cw], F32, tag="g2", name="g2")
                    V.scalar_tensor_tensor(out=g2, in0=Roff[(k2, j)],
                                           scalar=camv(C_R + 3 * i + k2),
                                           in1=g1, op0=OP.mult, op1=OP.add)
                    Hij = tmp.tile([P, cw], F32, tag=f"H{i}{j}", name=f"H{i}{j}")
                    V.tensor_mul(Hij, g2, scl_t[:, :, j])
                    H[i][j] = Hij

            cov_t = outs.tile([P, cw, 9], BF16, tag="cov", name="cov")
            p0, p1, p2, s01 = tp("p0"), tp("p1"), tp("p2"), tp("s01")
            # diagonal: sum of squares of H row i
            for i in range(3):
                for m, pm in ((0, p0), (1, p1), (2, p2)):
                    if lastc:
                        V.tensor_mul(pm, H[i][m], H[i][m])
                    else:
                        A.activation(out=pm, in_=H[i][m], func=AF.Square)
                EG.tensor_add(s01, p0, p1)
                EG.tensor_add(cov_t[:, :, 4 * i], s01, p2)
            # off-diagonal
            pe = EG if COVOFF_ON_G else V
            pe_alt = {0: pe, 1: G if lastc else pe, 2: pe}
            for ci2, (i, l) in enumerate(((0, 1), (0, 2), (1, 2))):
                pex = pe_alt[ci2]
                pex.tensor_mul(p0, H[i][0], H[l][0])
                pex.tensor_mul(p1, H[i][1], H[l][1])
                pex.tensor_mul(p2, H[i][2], H[l][2])
                pex.tensor_add(s01, p0, p1)
                pex.tensor_add(cov_t[:, :, 3 * i + l], s01, p2)
                if lastc:
                    V.tensor_copy(out=cov_t[:, :, 3 * l + i],
                                  in_=cov_t[:, :, 3 * i + l])
                else:
                    A.copy(out=cov_t[:, :, 3 * l + i], in_=cov_t[:, :, 3 * i + l])

            # means
            dx, dy = tp("dx"), tp("dy")
            V.tensor_scalar(out=dx, in0=t_crd[:, cc, 0], scalar1=camv(C_AX),
                            scalar2=camv(C_BX), op0=OP.mult, op1=OP.add)
            V.tensor_scalar(out=dy, in0=t_crd[:, cc, 1], scalar1=camv(C_AY),
                            scalar2=camv(C_BY), op0=OP.mult, op1=OP.add)
            sx, sy, dd = tp("sx"), tp("sy"), tp("dd")
            A.activation(out=sx, in_=dx, func=AF.Square)
            A.activation(out=sy, in_=dy, func=AF.Square)
            V.tensor_add(dd, sx, sy)
            nrm, rr, dsc = tp("nrm"), tp("rr"), tp("dsc")
            A.activation(out=nrm, in_=dd, func=AF.Sqrt, bias=1.0)
            V.reciprocal_approx_fast(out=rr, in_=nrm)
            V.tensor_mul(dsc, rr, t_dep[:, cc])

            mean_t = outs.tile([P, cw, 3], BF16, tag="mean", name="mean")
            for i in range(3):
                w1 = chain.tile([P, cw], F32, tag="w1", name="w1")
                A.activation(out=w1, in_=dx, func=AF.Copy,
                             scale=camv(C_R + 3 * i))
                w2 = chain.tile([P, cw], F32, tag="w2", name="w2")
                V.scalar_tensor_tensor(out=w2, in0=dy,
                                       scalar=camv(C_R + 3 * i + 1), in1=w1,
                                       op0=OP.mult, op1=OP.add)
                w3 = chain.tile([P, cw], F32, tag="w3", name="w3")
                V.scalar_tensor_tensor(out=w3, in0=w2,
                                       scalar=camv(C_R + 3 * i + 2), in1=dsc,
                                       op0=OP.add, op1=OP.mult)
                V.tensor_scalar(out=mean_t[:, :, i], in0=w3,
                                scalar1=camv(C_T + i), scalar2=None,
                                op0=OP.add)

            SO.dma_start(out=o_mean[:, cc, :], in_=mean_t[:])
            SO.dma_start(out=o_cov[:, cc, :], in_=cov_t[:])
            SO.dma_start(out=o_scl[:, cc, :], in_=scl_t[:])
            SO.dma_start(out=o_rot[:, cc, :], in_=rot_t[:])

        geom_emit = geometry_chunk

        # ---------------- phase 1: stream AoS blocks ----------------
        # emit all input DMAs first so the (FIFO) DMA ring front-loads them
        Tg_l, Ts_l = [], []
        for b in range(NBLK):
            cs = slice(b * W, (b + 1) * W)
            Tg = rawp.tile([P, W, 7], F32, tag="Tg", name="Tg")
            S.dma_start(out=Tg[:], in_=rawg[:, cs, :])
            Ts = rawp.tile([P, W, D_SH], BF16, tag="Ts", name="Ts")
            S.dma_start(out=Ts[:], in_=raws[:, cs, :])
            Tg_l.append(Tg)
            Ts_l.append(Ts)
        for b in range(NBLK):
            cs = slice(b * W, (b + 1) * W)
            h = next(i for i, (c0, cw) in enumerate(CHUNKS)
                     if c0 <= b * W < c0 + cw)
            c0 = CHUNKS[h][0]
            ls = slice(b * W - c0, (b + 1) * W - c0)
            Tg, Ts = Tg_l[b], Ts_l[b]

            sh_t = shp.tile([P, W, D_SH], BF16, tag="sh", name="sh")
            # 2-record (150-elem) periodicity keeps every inner run 4B-aligned
            # so the DVE 2x bf16 mode engages.
            mask_b = bass.AP(
                tensor=m_ap.tensor, offset=m_ap.offset,
                ap=[m_ap.ap[0], [0, W // 2], [1, 2 * D_SH]],
            )
            ts_ap = Ts[:]
            in0_b = bass.AP(tensor=ts_ap.tensor, offset=ts_ap.offset,
                            ap=[ts_ap.ap[0], [2 * D_SH, W // 2], [1, 2 * D_SH]])
            out_b = sh_t[:]
            out_b = bass.AP(tensor=out_b.tensor, offset=out_b.offset,
                            ap=[out_b.ap[0], [2 * D_SH, W // 2], [1, 2 * D_SH]])
            sh_eng = G if b in SH_G_BLOCKS else V
            sh_eng.tensor_tensor(out=out_b, in0=in0_b, in1=mask_b, op=OP.mult)
            S.dma_start(out=o_sh[:, cs, :], in_=sh_t[:])

            A.copy(out=q_h[h][:, ls, :], in_=Tg[:, :, 3:7])
            A.activation(out=sg_h[h][:, ls, :], in_=Tg[:, :, 0:3], func=AF.Sigmoid)
            if b == 7:
                geom_emit(0)
                geom_emit(1)
                geom_emit(2)



    nc.compile()
    return nc


_NC = None


def _get_nc():
    global _NC
    if _NC is None:
        _NC = _build()
    return _NC


def _sh_mask75():
    m = np.ones(25, np.float32)
    for dg in range(1, 5):
        m[dg * dg:(dg + 1) * (dg + 1)] = np.float32(0.1 * 0.25 ** dg)
    return np.tile(m, 3)


def kernel(extrinsics, intrinsics, coordinates, depths, opacities,
           raw_gaussians, h=256, w=256, eps=1e-8):
    nc = _get_nc()
    hh, ww = int(h), int(w)
    ext = np.asarray(extrinsics, dtype=np.float32)
    K = np.asarray(intrinsics, dtype=np.float32)
    crd = np.asarray(coordinates, dtype=np.float32)
    dth = np.asarray(depths, dtype=np.float32)
    opa = np.asarray(opacities, dtype=np.float32)
    rg = np.asarray(raw_gaussians, dtype=np.float32)

    import ml_dtypes as _mld
    msk = np.broadcast_to(np.tile(_sh_mask75(), 2).astype(_mld.bfloat16),
                          (P, 2 * D_SH)).copy()

    import ml_dtypes
    rg_sh = rg[..., 7:].astype(ml_dtypes.bfloat16)
    in_maps = []
    for c in range(8):
        b, v = c // 2, c % 2
        Kb = K[b, v, 0].astype(np.float64)
        fx, fy, cx, cy = Kb[0, 0], Kb[1, 1], Kb[0, 2], Kb[1, 2]
        mult = 0.1 * (1.0 / (fx * ww) + 1.0 / (fy * hh))
        Rm = ext[b, v, 0, :3, :3].astype(np.float64)
        t = ext[b, v, 0, :3, 3].astype(np.float64)
        camv = np.concatenate([
            [1.0 / fx, -cx / fx, 1.0 / fy, -cy / fy, 14.5 * mult, 0.5 * mult],
            Rm.flatten(), t, -Rm.flatten(),
        ]).astype(np.float32)
        in_maps.append({
            "rawg": np.ascontiguousarray(rg[b, v, :, :7]).reshape(P, COLS, 7),
            "raws": np.ascontiguousarray(rg_sh[b, v]).reshape(P, COLS, D_SH),
            "crd": np.ascontiguousarray(crd[b, v]).reshape(P, COLS, 2),
            "dep": np.ascontiguousarray(dth[b, v]).reshape(P, COLS),
            "cam": np.broadcast_to(camv, (P, NCAM)).copy(),
            "msk": msk,
        })

    res = run_bass_kernel_spmd(nc, in_maps, list(range(8))).results

    def gather(name, klast):
        return np.stack([res[c][name].reshape(65536, klast).astype(np.float32)
                         for c in range(8)]).reshape(4, 2, 65536, klast)

    means = gather("o_mean", 3)
    cov = gather("o_cov", 9).reshape(4, 2, 65536, 3, 3)
    scales = gather("o_scl", 3)
    rotations = gather("o_rot", 4)
    sh = np.stack([res[c]["o_sh"].reshape(65536, D_SH).astype(np.float32)
                   for c in range(8)]).reshape(4, 2, 65536, 3, 25)
    return means, cov, scales, rotations, sh, opa
